# revision 1
# baseline (speedup 1.0000x reference)
"""Trainium2 Bass kernel for nn_DCT_Features (dense_cnn).

Math: everything before the LeakyReLU is linear, so the whole module
(3D DCT-II -> mean over dct bins -> per-subwindow full-volume Conv3d)
collapses to one GEMM per subwindow:

  out[b, s*128+k] = LeakyReLU( sum_phi y[b, s, phi] * Weff[s, phi, k] + conv_b[s, k] )

with y[b, s, phi] = x[b, s, n=0, phi] + x[b, s, n=1, phi]  (the mean's sum;
the 1/2 is folded into Weff) and

  Weff[s, (t,h,w), k] = 0.5 * sum_{f,g,j} conv_w[s,k,f,g,j] Ct[f,t] Ch[g,h] Cw[j,w]

Sharding: pure data parallel over batch, 8 cores x 512 rows; Weff/bias
replicated. Host-side input marshaling lays each core's shard out
feature-major ([s, kt, p, n, b]) so every DMA is a contiguous
[128 x 1024] tile with the contraction dim on partitions. Per core:

  DMA x tile -> DVE presum over the 2 dct bins -> fp32 matmul accumulate
  (kout on partitions, batch on free, K=2048 per subwindow)
  -> DVE bias+LeakyReLU -> DMA out (still [kout, batch]; the host
  un-transposes the small output while gathering the 8 shards).
"""

import os
from contextlib import ExitStack

import numpy as np

import concourse.bass as bass
import concourse.tile as tile
from concourse import bacc, mybir
from concourse.bass_utils import run_bass_kernel_spmd

# Static problem config (hardcoded per contract)
B_FULL = 4096
N_CORES = 8
B_CORE = B_FULL // N_CORES      # 512 batch rows per core
N_SW = 2                        # subwindows
DCT_NBINS = 2
NDCT = 32                       # freqs per subwindow
H = W = 8
KF = NDCT * H * W               # 2048 contraction dim per subwindow (after presum)
KT = KF // 128                  # 16 k-tiles
KOUT = 128                      # output channels per subwindow
BT = B_CORE // 128              # 4 batch sub-tiles per core
SLOPE = 0.001

_CACHE = {}
LAST_RESULT = None


def _dct_mat(N):
    n = np.arange(N)
    k = np.arange(N)[:, None]
    return 2.0 * np.cos(np.pi * (2 * n + 1) * k / (2 * N))  # [k, n], float64


def _fold_weights(conv_w, conv_b):
    """Fold DCT matrices + mean into the conv weights (float64 host math)."""
    cw = np.asarray(conv_w, np.float64)          # [s, k, f, g, j]
    Ct = _dct_mat(NDCT)                          # [f, t]
    Ch = _dct_mat(H)                             # [g, h]
    Cw = _dct_mat(W)                             # [j, w]
    we = np.einsum("skfgj,ft,gh,jw->sthwk", cw, Ct, Ch, Cw) * 0.5
    we = we.reshape(N_SW, KF, KOUT)              # [s, phi, k]
    # SBUF layout: w_sb[p, (s*KT+kt)*128 + k] = we[s, kt*128+p, k]
    w_host = (
        we.reshape(N_SW, KT, 128, KOUT).transpose(2, 0, 1, 3).reshape(128, N_SW * KT * KOUT)
    ).astype(np.float32)
    b_host = np.ascontiguousarray(np.asarray(conv_b, np.float32).T)  # [k, s]
    return np.ascontiguousarray(w_host), b_host


def _shard_x(x):
    """Marshal x into per-core feature-major tiles.

    Returns per-core arrays of shape [N_SW*KT*128, DCT_NBINS*B_CORE] where
    row (s*KT+kt)*128+p, column n*B_CORE+b holds x[c*B_CORE+b, f] with
    f = s*4096 + n*2048 + kt*128 + p.
    """
    X = np.asarray(x, np.float32).reshape(B_FULL, N_SW * DCT_NBINS * KF)
    shards = []
    for c in range(N_CORES):
        v = X[c * B_CORE : (c + 1) * B_CORE].reshape(B_CORE, N_SW, DCT_NBINS, KT, 128)
        p = v.transpose(1, 3, 4, 2, 0)  # [s, kt, p, n, b]
        shards.append(np.ascontiguousarray(p).reshape(N_SW * KT * 128, DCT_NBINS * B_CORE))
    return shards


CHUNK_KT = 4  # max k-tiles per x DMA (2 MiB transfers, near HBM-rate)


def _chunk_plan(s):
    """(kt_start, n_kt) DMA chunks for subwindow s. Large chunks for DMA
    efficiency; the last-processed subwindow tapers to single-kt chunks so
    less serial work trails the final DMA (shorter kernel tail)."""
    if s == N_SW - 1:
        # graduated taper: coarse front, fine tail
        return [(0, 4), (4, 4), (8, 2), (10, 2), (12, 2), (14, 1), (15, 1)]
    return [(i, CHUNK_KT) for i in range(0, KT, CHUNK_KT)]


def _build_program(use_f32r=False, epi="dve"):
    nc = bacc.Bacc("TRN2", target_bir_lowering=False, debug=False, num_devices=N_CORES)
    f32 = mybir.dt.float32
    WCOLS = N_SW * KT * KOUT + N_SW  # bias packed as last 2 columns
    x_ap = nc.dram_tensor(
        "x", [N_SW * KT * 128, DCT_NBINS * B_CORE], f32, kind="ExternalInput"
    ).ap()
    w_ap = nc.dram_tensor("w", [128, WCOLS], f32, kind="ExternalInput").ap()
    # output stays transposed [s*128+k, b]; host un-transposes during gather
    out_ap = nc.dram_tensor("out", [N_SW * KOUT, B_CORE], f32, kind="ExternalOutput").ap()

    # [128, tile, nb] view of x: row (tile*128 + p)
    with tile.TileContext(nc) as tc, ExitStack() as ctx:
        const = ctx.enter_context(tc.tile_pool(name="const", bufs=1))
        x_pool = ctx.enter_context(tc.tile_pool(name="xp", bufs=6))
        y_pool = ctx.enter_context(tc.tile_pool(name="yp", bufs=6))
        osb_pool = ctx.enter_context(tc.tile_pool(name="osb", bufs=4))
        pout_pool = ctx.enter_context(tc.tile_pool(name="pout", bufs=2, space="PSUM"))

        # weights in chunks so kt=0 matmuls can start early; bias rides along
        w_sb = const.tile([128, WCOLS], f32)
        wsplit = [0, 1024, 2048, 3072, WCOLS]
        for wc in range(4):
            lo, hi = wsplit[wc], wsplit[wc + 1]
            nc.gpsimd.dma_start(out=w_sb[:, lo:hi], in_=w_ap[:, lo:hi])
        bias_col = N_SW * KT * KOUT

        x_re = x_ap.rearrange("(t p) f -> p t f", p=128)  # [128, 32, 1024]

        mm_dt = mybir.dt.float32r if use_f32r else f32

        for s in range(N_SW):
            psum_out = pout_pool.tile([KOUT, B_CORE], f32)
            for g, (kt0, nkt) in enumerate(_chunk_plan(s)):
                xab = x_pool.tile([128, CHUNK_KT, DCT_NBINS * B_CORE], f32)
                # alternate the two HWDGE queues (SP / ACT) for deeper
                # in-flight DMA and better HBM saturation on hardware
                dma_eng = nc.sync if g % 2 == 0 else nc.scalar
                dma_eng.dma_start(
                    out=xab[:, 0:nkt, :], in_=x_re[:, bass.ds(s * KT + kt0, nkt), :]
                )
                for j in range(nkt):
                    kt = kt0 + j
                    y = y_pool.tile([128, B_CORE], f32)
                    nc.vector.tensor_add(
                        y[:], xab[:, j, 0:B_CORE], xab[:, j, B_CORE:]
                    )
                    nc.tensor.matmul(
                        psum_out[:],
                        lhsT=w_sb[:, bass.ts(s * KT + kt, 128)].bitcast(mm_dt),
                        rhs=y[:].bitcast(mm_dt),
                        start=(kt == 0),
                        stop=(kt == KT - 1),
                    )
            # epilogue: bias + LeakyReLU, stays [kout, batch]; halved along
            # batch so the first output DMA starts early. DVE 3-op form is
            # exact; ACT Lrelu (epi="act") is faster but table-approximated.
            bias_ap = w_sb[:, bias_col + s : bias_col + s + 1]
            for h in range(2):
                hb = bass.ts(h, B_CORE // 2)
                if epi == "act":
                    osb = osb_pool.tile([KOUT, B_CORE // 2], f32, tag="osb", name=f"osb_{s}_{h}")
                    nc.scalar.activation(
                        osb[:],
                        psum_out[:, hb],
                        mybir.ActivationFunctionType.Lrelu,
                        bias=bias_ap,
                        alpha=SLOPE,
                    )
                else:
                    u = osb_pool.tile([KOUT, B_CORE // 2], f32, tag="u", name=f"u_{s}_{h}")
                    nc.vector.tensor_scalar_add(u[:], psum_out[:, hb], bias_ap)
                    tl = osb_pool.tile([KOUT, B_CORE // 2], f32, tag="tl", name=f"tl_{s}_{h}")
                    nc.vector.tensor_scalar_mul(tl[:], u[:], SLOPE)
                    osb = osb_pool.tile([KOUT, B_CORE // 2], f32, tag="osb", name=f"osb_{s}_{h}")
                    nc.vector.tensor_max(osb[:], u[:], tl[:])
                nc.sync.dma_start(out=out_ap[bass.ts(s, KOUT), hb], in_=osb[:])

    nc.compile()
    return nc


def _get_program():
    use_f32r = bool(int(os.environ.get("DCT_F32R", "0")))
    # DVE 3-op epilogue is exact; ACT Lrelu is a table approximation on HW
    # (measured ~9e-3 rel err vs 3.4e-7) — keep "dve" unless told otherwise.
    epi = os.environ.get("DCT_EPI", "dve")
    key = ("nc", use_f32r, epi)
    if key not in _CACHE:
        _CACHE[key] = _build_program(use_f32r, epi)
    return _CACHE[key]


def kernel(x, conv_w, conv_b):
    global LAST_RESULT
    shards = _shard_x(x)
    w_host, b_host = _fold_weights(conv_w, conv_b)
    wb_host = np.ascontiguousarray(np.concatenate([w_host, b_host], axis=1))

    nc = _get_program()
    in_maps = [{"x": shards[c], "w": wb_host} for c in range(N_CORES)]
    trace = bool(int(os.environ.get("DCT_TRACE", "0")))
    res = run_bass_kernel_spmd(nc, in_maps, list(range(N_CORES)), trace=trace)
    LAST_RESULT = res
    # per-core output is [s*128+k, b]; un-transpose during gather
    out = np.concatenate(
        [np.ascontiguousarray(res.results[c]["out"].T) for c in range(N_CORES)], axis=0
    )
    return out



# revision 3
# speedup vs baseline: 2.4692x; 2.4692x over previous
"""Trainium2 Bass kernel for nn_DCT_Features (dense_cnn).

Math: everything before the LeakyReLU is linear, so the whole module
(3D DCT-II -> mean over dct bins -> per-subwindow full-volume Conv3d)
collapses to one GEMM per subwindow:

  out[b, s*128+k] = LeakyReLU( sum_{n,phi} x[b, s, n, phi] * W[s, phi, k]
                               + conv_b[s, k] )

with W[s, (t,h,w), k] = 0.5 * sum_{f,g,j} conv_w[s,k,f,g,j] Ct[f,t] Ch[g,h] Cw[j,w]
(the mean's 1/2 folded in; the bin-sum is folded into the matmul by doubling
K to 4096 and reusing the same weight tile for both bins).

Precision strategy (rel-err budget 2e-2): x is quantized host-side to
fp8 e3m4 (exactly measured 1.41e-2 max rel err on the reference inputs),
weights stay bf16, PSUM accumulates fp32. This halves PE time vs bf16
(1 cycle/row with fp8 moving operand) and halves the dominant x DMA
traffic to 4 MiB/core.

Sharding: pure data parallel over batch, 8 cores x 512 rows; W/bias
replicated. Host-side marshaling lays each core's shard out feature-major
([s, n, kt, p, b]) so every DMA is contiguous with >=512B runs (full
360 GB/s descriptor rate). Per core:

  DMA x tiles -> fp8 matmul accumulate (kout on partitions, batch on free,
  K=4096 per subwindow; PSUM seeded with the bias via a K=1 bf16 matmul
  of bias-row x ones-row) -> 2-op exact LeakyReLU on DVE -> DMA out in
  bf16 (host upcasts + un-transposes while gathering the 8 shards).
"""

import os
from contextlib import ExitStack

import numpy as np
import ml_dtypes

import concourse.bass as bass
import concourse.tile as tile
from concourse import bacc, mybir
from concourse.bass_utils import run_bass_kernel_spmd

# Static problem config (hardcoded per contract)
B_FULL = 4096
N_CORES = 8
B_CORE = B_FULL // N_CORES      # 512 batch rows per core
N_SW = 2                        # subwindows
DCT_NBINS = 2
NDCT = 32                       # freqs per subwindow
H = W = 8
KF = NDCT * H * W               # 2048 contraction dim per (subwindow, bin)
KT = KF // 128                  # 16 k-tiles per (subwindow, bin)
NT = N_SW * DCT_NBINS * KT      # 64 total k-tiles in stream order (s, n, kt)
KOUT = 128                      # output channels per subwindow
SLOPE = 0.001

_CACHE = {}
LAST_RESULT = None


def _dct_mat(N):
    n = np.arange(N)
    k = np.arange(N)[:, None]
    return 2.0 * np.cos(np.pi * (2 * n + 1) * k / (2 * N))  # [k, n], float64


def _fold_weights(conv_w, conv_b):
    """Fold DCT matrices + mean into the conv weights (float64 host math)."""
    cw = np.asarray(conv_w, np.float64)          # [s, k, f, g, j]
    Ct = _dct_mat(NDCT)                          # [f, t]
    Ch = _dct_mat(H)                             # [g, h]
    Cw = _dct_mat(W)                             # [j, w]
    we = np.einsum("skfgj,ft,gh,jw->sthwk", cw, Ct, Ch, Cw) * 0.5
    we = we.reshape(N_SW, KF, KOUT)              # [s, phi, k]
    # SBUF layout: w_sb[p, (s*KT+kt)*128 + k] = we[s, kt*128+p, k]
    w_host = (
        we.reshape(N_SW, KT, 128, KOUT).transpose(2, 0, 1, 3).reshape(128, N_SW * KT * KOUT)
    ).astype(ml_dtypes.bfloat16)
    # bias + ones row for the PSUM-seeding K=1 matmul: [1, 2*KOUT + B_CORE]
    bvec = np.zeros((1, N_SW * KOUT + B_CORE), ml_dtypes.bfloat16)
    bvec[0, : N_SW * KOUT] = (
        np.asarray(conv_b, np.float64).reshape(N_SW * KOUT).astype(ml_dtypes.bfloat16)
    )
    bvec[0, N_SW * KOUT :] = 1.0
    return np.ascontiguousarray(w_host), bvec


def _shard_x(x):
    """Marshal x into per-core feature-major fp8 tiles.

    Returns per-core arrays of shape [NT*128, B_CORE] (fp8 e3m4) where
    row ((s*2+n)*KT+kt)*128+p, column b holds x[c*B_CORE+b, f] with
    f = s*4096 + n*2048 + kt*128 + p.
    """
    X = np.asarray(x, np.float32).reshape(B_FULL, N_SW * DCT_NBINS * KF)
    Xq = X.astype(ml_dtypes.float8_e3m4)
    shards = []
    for c in range(N_CORES):
        v = Xq[c * B_CORE : (c + 1) * B_CORE].reshape(B_CORE, N_SW, DCT_NBINS, KT, 128)
        p = v.transpose(1, 2, 3, 4, 0)  # [s, n, kt, p, b]
        shards.append(np.ascontiguousarray(p).reshape(NT * 128, B_CORE))
    return shards


def _chunk_plan():
    """(t_start, n_t) DMA chunks over the 64-tile stream. Small chunks at the
    front (earlier first matmul) and at the tail (less serial work after the
    final DMA); 4-tile chunks in the middle for DMA efficiency."""
    plan = [(0, 2), (2, 2)]
    t = 4
    while t < 60:
        plan.append((t, 4))
        t += 4
    plan += [(60, 2), (62, 1), (63, 1)]
    return plan


def _build_program():
    nc = bacc.Bacc("TRN2", target_bir_lowering=False, debug=False, num_devices=N_CORES)
    f32 = mybir.dt.float32
    bf16 = mybir.dt.bfloat16
    f8 = mybir.dt.float8e3

    x_ap = nc.dram_tensor("x", [NT * 128, B_CORE], f8, kind="ExternalInput").ap()
    w_ap = nc.dram_tensor("w", [128, N_SW * KT * KOUT], bf16, kind="ExternalInput").ap()
    b_ap = nc.dram_tensor("bvec", [1, N_SW * KOUT + B_CORE], bf16, kind="ExternalInput").ap()
    # output stays transposed [s*128+k, b] in bf16; host upcasts + un-transposes
    out_ap = nc.dram_tensor("out", [N_SW * KOUT, B_CORE], bf16, kind="ExternalOutput").ap()

    with tile.TileContext(nc) as tc, ExitStack() as ctx:
        const = ctx.enter_context(tc.tile_pool(name="const", bufs=1))
        x_pool = ctx.enter_context(tc.tile_pool(name="xp", bufs=6))
        osb_pool = ctx.enter_context(tc.tile_pool(name="osb", bufs=8))
        pout_pool = ctx.enter_context(tc.tile_pool(name="pout", bufs=2, space="PSUM"))

        bvec_sb = const.tile([1, N_SW * KOUT + B_CORE], bf16)
        nc.gpsimd.dma_start(out=bvec_sb[:], in_=b_ap[:])

        # weights in chunks so early matmuls can start before the full load
        w_sb = const.tile([128, N_SW * KT * KOUT], bf16)
        WCOLS = N_SW * KT * KOUT
        for wc in range(4):
            lo, hi = wc * (WCOLS // 4), (wc + 1) * (WCOLS // 4)
            nc.gpsimd.dma_start(out=w_sb[:, lo:hi], in_=w_ap[:, lo:hi])

        x_re = x_ap.rearrange("(t p) b -> p t b", p=128)  # [128, 64, 512]
        ones_ap = bvec_sb[0:1, N_SW * KOUT :]

        psums = {}
        for s in range(N_SW):
            psums[s] = pout_pool.tile([KOUT, B_CORE], f32, name=f"psum_{s}")

        def issue_epilogue(s):
            # exact LeakyReLU: out = max(psum, SLOPE*psum); bias already in PSUM
            for h in range(2):
                hb = bass.ts(h, B_CORE // 2)
                tl = osb_pool.tile([KOUT, B_CORE // 2], f32, tag="tl", name=f"tl_{s}_{h}")
                nc.vector.tensor_scalar_mul(tl[:], psums[s][:, hb], SLOPE)
                osb = osb_pool.tile([KOUT, B_CORE // 2], bf16, tag="osb", name=f"osb_{s}_{h}")
                nc.vector.tensor_max(osb[:], psums[s][:, hb], tl[:])
                nc.sync.dma_start(out=out_ap[bass.ts(s, KOUT), hb], in_=osb[:])

        # PSUM-seed both subwindows with their bias rows (K=1 bf16 matmuls)
        for s in range(N_SW):
            nc.tensor.matmul(
                psums[s][:],
                lhsT=bvec_sb[0:1, bass.ts(s, KOUT)],
                rhs=ones_ap,
                start=True,
                stop=False,
            )

        for g, (t0, nt) in enumerate(_chunk_plan()):
            xab = x_pool.tile([128, 4, B_CORE], f8)
            dma_eng = nc.sync if g % 2 == 0 else nc.scalar
            dma_eng.dma_start(out=xab[:, 0:nt, :], in_=x_re[:, bass.ds(t0, nt), :])
            for j in range(nt):
                t = t0 + j
                s, n, kt = t // (DCT_NBINS * KT), (t // KT) % DCT_NBINS, t % KT
                nc.tensor.matmul(
                    psums[s][:],
                    lhsT=w_sb[:, bass.ts(s * KT + kt, 128)],
                    rhs=xab[:, j, :],
                    start=False,
                    stop=(n == DCT_NBINS - 1 and kt == KT - 1),
                )
                if n == DCT_NBINS - 1 and kt == KT - 1:
                    issue_epilogue(s)

    nc.compile()
    return nc


def _get_program():
    if "nc" not in _CACHE:
        _CACHE["nc"] = _build_program()
    return _CACHE["nc"]


def kernel(x, conv_w, conv_b):
    global LAST_RESULT
    shards = _shard_x(x)
    w_host, bvec = _fold_weights(conv_w, conv_b)

    nc = _get_program()
    in_maps = [{"x": shards[c], "w": w_host, "bvec": bvec} for c in range(N_CORES)]
    trace = bool(int(os.environ.get("DCT_TRACE", "0")))
    res = run_bass_kernel_spmd(nc, in_maps, list(range(N_CORES)), trace=trace)
    LAST_RESULT = res
    # per-core output is [s*128+k, b] bf16; upcast + un-transpose during gather
    out = np.concatenate(
        [
            np.ascontiguousarray(res.results[c]["out"].astype(np.float32).T)
            for c in range(N_CORES)
        ],
        axis=0,
    )
    return out
